# revision 3
# baseline (speedup 1.0000x reference)
"""ChannelAttention Trainium2 kernel (self-contained).

Problem: B=16, H=W=64 (N=4096 tokens), C=512, heads=8, d=64, fp32.
  qkv = x @ qkv_w (+bias);  q,k l2-normalized over tokens;
  attn = softmax((q*exp(scale))^T k);  out = attn @ v^T;  y = out @ proj_w + b.

Sharding: pure data-parallel, 2 batches per core on 8 cores. No collectives.

v2 fast path (zero qkv/proj biases — the graded instance):
  Channel attention only ever uses q,k through the Gram matrix
  (q^T k + the l2 norms on its diagonal), and the value/projection path
  is linear in x. Exploiting N >> C:
    XtX  = X^T X                      [C, C]   (one pass over tokens)
    G_h  = Wqk_h^T XtX Wqk_h          [128,128] per head == [q|k]^T [q|k]
    A_h  = softmax(norm-scaled G_qk)  [64, 64]
    M    = blockdiag(A_h) @ Wv^T      [C, C]
    Wf   = M^T @ Wp                   [C, C]
    y^T  = Wf^T X^T                   (one pass over tokens)
  Token-dimension work collapses to two C x C passes over x (XtX and
  y^T); everything else is tiny feature-space algebra. All matmuls in
  bf16 (relmax ~3e-3 vs 2e-2 gate), fp32 PSUM accumulation.

v1 path (general biases) kept as fallback: per-token qkv with the
Z=[q|k] Gram trick, fp32r matmuls.
"""

import os
import numpy as np

P = 128
C = 512
CCH = C // P            # 4 contraction chunks
HEADS = 8
NPAIR = HEADS // 2      # 4 head pairs
D = 64
EPS = 1.55e-5
N_CORES = 8

_CACHE = {}


def _pbroadcast(bass, ap, p):
    # read a [1, F] DRAM row with partition-step 0 -> broadcast to p partitions
    return bass.AP(tensor=ap.tensor, offset=ap.offset,
                   ap=[[0, p]] + [list(d) for d in ap.ap[1:]])


# ---------------------------------------------------------------------------
# v3: XtX / fused-projection path (zero biases)
# ---------------------------------------------------------------------------

def _build_v3(nb, n, es):
    """nb: batches per core; n: tokens per batch; es: 8 exp(scale) floats.

    v3 scheduling/DMA rework over v2:
      - y output in bf16, written on the second HWDGE queue (Act engine),
        so writes never contend with reads on the SP queue.
      - weights load on the Act queue at t=0; the SP queue starts with
        batch-0 x tiles so the XtX pass is never descriptor-starved.
      - x tiles paired in DRAM -> 2 KB per-partition DMA runs.
      - ~28 identity warm-up matmuls during the initial DMA window keep
        the PE HAM clock warm before real work lands.
      - batched norm/softmax mid-phase (one [64, 8*64] exp, 3D reduces,
        no max-subtraction: |logits| <= max(es)) to shorten the serial
        chain that previously starved the PE between phases.
    """
    from contextlib import ExitStack
    import concourse.bass as bass  # noqa: F401
    from concourse import bacc
    import concourse.mybir as mybir
    import concourse.tile as tile
    from concourse.masks import make_identity

    f32 = mybir.dt.float32
    bf16 = mybir.dt.bfloat16
    X = mybir.AxisListType.X
    AF = mybir.ActivationFunctionType

    nt2 = n // (2 * P)       # 16 paired token tiles per batch
    ng = n // 1024           # 4 token groups per batch (y^T pass)
    es_uniform = len(set(es)) == 1
    no_maxsub = max(es) <= 5.0

    nc = bacc.Bacc("TRN2", target_bir_lowering=False)

    xb_d = nc.dram_tensor("xb", [nb, nt2, P, 2, C], bf16, kind="ExternalInput")
    xt_d = nc.dram_tensor("xt", [nb, C, n], bf16, kind="ExternalInput")
    wqk_d = nc.dram_tensor("wqk", [P, CCH, 2 * C], bf16, kind="ExternalInput")
    wvt_d = nc.dram_tensor("wvt", [P, NPAIR, C], bf16, kind="ExternalInput")
    wp_d = nc.dram_tensor("wp", [P, NPAIR, C], bf16, kind="ExternalInput")
    y_d = nc.dram_tensor("y", [nb, C, n], bf16, kind="ExternalOutput")

    with tile.TileContext(nc) as tc, ExitStack() as ctx:
        consts = ctx.enter_context(tc.tile_pool(name="consts", bufs=1))
        xp = ctx.enter_context(tc.tile_pool(name="xp", bufs=6))
        xtp = ctx.enter_context(tc.tile_pool(name="xtp", bufs=2 * 4))
        xtxp = ctx.enter_context(tc.tile_pool(name="xtxp", bufs=2))
        t1p = ctx.enter_context(tc.tile_pool(name="t1p", bufs=2))
        gpool = ctx.enter_context(tc.tile_pool(name="gpool", bufs=2))
        smp = ctx.enter_context(tc.tile_pool(name="smp", bufs=4))
        tinp = ctx.enter_context(tc.tile_pool(name="tinp", bufs=4))
        atp = ctx.enter_context(tc.tile_pool(name="atp", bufs=2))
        mpool = ctx.enter_context(tc.tile_pool(name="mpool", bufs=2))
        wfp = ctx.enter_context(tc.tile_pool(name="wfp", bufs=2))
        ypool = ctx.enter_context(tc.tile_pool(name="ypool", bufs=6))
        pxtx = ctx.enter_context(tc.tile_pool(name="pxtx", bufs=4, space="PSUM"))
        pmid = ctx.enter_context(tc.tile_pool(name="pmid", bufs=4, space="PSUM"))

        # --- resident constants; weights ride the Act HWDGE queue so the
        # SP queue belongs to batch-0 x tiles from the first descriptor ---
        wqk_sb = consts.tile([P, CCH, 2 * C], bf16)
        nc.scalar.dma_start(wqk_sb[:], wqk_d[:])
        wvt_sb = consts.tile([P, NPAIR, C], bf16)
        nc.scalar.dma_start(wvt_sb[:], wvt_d[:])
        wp_sb = consts.tile([P, NPAIR, C], bf16)
        nc.scalar.dma_start(wp_sb[:], wp_d[:])
        ident = consts.tile([P, P], f32)
        make_identity(nc, ident[:])
        identb = consts.tile([P, P], bf16)
        nc.vector.tensor_copy(out=identb[:], in_=ident[:])
        ioff = consts.tile([P, D], f32)
        nc.gpsimd.memset(ioff[:], 0.0)
        nc.gpsimd.affine_select(
            out=ioff[:], in_=ioff[:], compare_op=mybir.AluOpType.not_equal,
            fill=1.0, base=-D, pattern=[[-1, D]], channel_multiplier=1,
        )
        ones64 = consts.tile([D, D], f32)
        nc.gpsimd.memset(ones64[:], 1.0)

        # HAM warm-up: keep the PE busy while the first x tiles stream in.
        pmw = pmid.tile([P, P], f32, tag="pm", name="warm")
        for _ in range(28):
            nc.tensor.matmul(pmw[:], identb[:], identb[:],
                             start=True, stop=True)

        state = [dict(xt_ts=[]) for _ in range(nb)]

        def xt_load(b, gi):
            xt_t = xtp.tile([P, CCH, 1024], bf16, tag="xt",
                            name=f"xtt{b}_{gi}")
            nc.sync.dma_start(
                out=xt_t[:],
                in_=xt_d[b].rearrange("(co ci) n -> ci co n", ci=P)
                [:, :, gi * 1024:(gi + 1) * 1024])
            state[b]["xt_ts"].append(xt_t)

        def gen_A(b):
            """XtX accumulation over paired token tiles. For b>=1 also
            emits the previous batch's x^T group loads (so they follow
            this batch's x tiles on the SP queue)."""
            st = state[b]
            xtx_ps = [pxtx.tile([P, C], f32, tag="xtx", name=f"xtx{b}_{cb}")
                      for cb in range(CCH)]
            st["xtx_ps"] = xtx_ps
            for t in range(nt2):
                x_t = xp.tile([P, 2, C], bf16, tag="x", name=f"x{b}_{t}")
                nc.sync.dma_start(out=x_t[:], in_=xb_d[b, t])
                for sub in range(2):
                    for cb in range(CCH):
                        nc.tensor.matmul(
                            xtx_ps[cb][:, cb * P:],
                            x_t[:, sub, cb * P:(cb + 1) * P],
                            x_t[:, sub, cb * P:],
                            start=(t == 0 and sub == 0),
                            stop=(t == nt2 - 1 and sub == 1))
                    yield
                if b >= 1 and t >= 2 and (t - 2) % 4 == 0:
                    gi = (t - 2) // 4
                    if gi < ng:
                        xt_load(b - 1, gi)

        def gen_MID(b):
            """xtx evict -> T1 -> G -> batched norms -> batched softmax
            -> M -> Wf. Yield values are filler-step budgets."""
            st = state[b]
            xtx_ps = st["xtx_ps"]
            xtx_sb = xtxp.tile([P, CCH, C], bf16, tag="xtx", name=f"xtxsb{b}")
            for cb in range(CCH):
                if cb % 2 == 0:
                    nc.vector.tensor_copy(out=xtx_sb[:, cb, cb * P:],
                                          in_=xtx_ps[cb][:, cb * P:])
                else:
                    nc.scalar.copy(out=xtx_sb[:, cb, cb * P:],
                                   in_=xtx_ps[cb][:, cb * P:])
            yield 0.5
            for i, j in ((1, 0), (2, 0), (2, 1), (3, 0), (3, 1), (3, 2)):
                ptt = pmid.tile([P, P], bf16, tag="pm", name=f"ptt{b}_{i}{j}")
                nc.tensor.transpose(
                    ptt[:], xtx_sb[:, j, i * P:(i + 1) * P], identb[:])
                if (i + j) % 2 == 0:
                    nc.vector.tensor_copy(
                        out=xtx_sb[:, i, j * P:(j + 1) * P], in_=ptt[:])
                else:
                    nc.scalar.copy(
                        out=xtx_sb[:, i, j * P:(j + 1) * P], in_=ptt[:])
            yield 0.5
            # T1 = XtX @ Wqk   [C, 1024]
            t1_sb = t1p.tile([P, CCH, 2 * C], bf16, tag="t1", name=f"t1sb{b}")
            for c1b in range(CCH):
                pA = pmid.tile([P, C], f32, tag="pm", name=f"t1a{b}_{c1b}")
                pB = pmid.tile([P, C], f32, tag="pm", name=f"t1b{b}_{c1b}")
                for c2b in range(CCH):
                    st_ap = xtx_sb[:, c2b, c1b * P:(c1b + 1) * P]
                    nc.tensor.matmul(pA[:], st_ap, wqk_sb[:, c2b, 0:C],
                                     start=(c2b == 0), stop=(c2b == CCH - 1))
                    nc.tensor.matmul(pB[:], st_ap, wqk_sb[:, c2b, C:2 * C],
                                     start=(c2b == 0), stop=(c2b == CCH - 1))
                nc.vector.tensor_copy(out=t1_sb[:, c1b, 0:C], in_=pA[:])
                nc.scalar.copy(out=t1_sb[:, c1b, C:2 * C], in_=pB[:])
                yield 0.4
            # G_h = Wqk_h^T T1_h  [128, 128] per head, into one SBUF tile;
            # diag extraction follows each eviction on gpsimd.
            g_all = gpool.tile([P, HEADS, P], f32, tag="g", name=f"g{b}")
            dt_all = gpool.tile([P, HEADS, P], f32, tag="dt", name=f"dt{b}")
            for h in range(HEADS):
                pg = pmid.tile([P, P], f32, tag="pm", name=f"pg{b}_{h}")
                for c1b in range(CCH):
                    nc.tensor.matmul(
                        pg[:], wqk_sb[:, c1b, h * P:(h + 1) * P],
                        t1_sb[:, c1b, h * P:(h + 1) * P],
                        start=(c1b == 0), stop=(c1b == CCH - 1))
                if h % 2 == 0:
                    nc.vector.tensor_copy(out=g_all[:, h, :], in_=pg[:])
                else:
                    nc.scalar.copy(out=g_all[:, h, :], in_=pg[:])
                nc.gpsimd.tensor_mul(dt_all[:, h, :], g_all[:, h, :],
                                     ident[:])
                if h % 2 == 1:
                    yield 0.5
            # batched inverse norms: rs[:, h] = es-scaled rsqrt(max(.,EPS))
            rs = smp.tile([P, HEADS], f32, tag="rs", name=f"rs{b}")
            nc.vector.tensor_reduce(out=rs[:], in_=dt_all[:],
                                    op=mybir.AluOpType.add, axis=X)
            nc.vector.tensor_scalar_max(out=rs[:], in0=rs[:], scalar1=EPS)
            srt = smp.tile([P, HEADS], f32, tag="srt", name=f"srt{b}")
            nc.scalar.activation(out=srt[:], in_=rs[:], func=AF.Sqrt)
            nc.vector.reciprocal(out=rs[:], in_=srt[:])
            if es_uniform:
                if es[0] != 1.0:
                    nc.vector.tensor_scalar_mul(
                        out=rs[0:D, :], in0=rs[0:D, :], scalar1=es[0])
            else:
                for h in range(HEADS):
                    nc.gpsimd.tensor_scalar_mul(
                        out=rs[0:D, h:h + 1], in0=rs[0:D, h:h + 1],
                        scalar1=es[h])
            yield 2.0
            # batched logits for all heads: pa_all[:, h, :] = scaled block
            rsq = smp.tile([D, HEADS, D], f32, tag="rsq", name=f"rsq{b}")
            dskall = smp.tile([P, HEADS, D], f32, tag="dsk", name=f"dsk{b}")
            for h in range(HEADS):
                nc.gpsimd.tensor_scalar_mul(
                    out=rsq[:, h, :], in0=ones64[:], scalar1=rs[0:D, h:h + 1])
                nc.gpsimd.tensor_scalar_mul(
                    out=dskall[D:P, h, :], in0=ioff[D:P, :],
                    scalar1=rs[D:P, h:h + 1])
            pa_all = pmid.tile([D, HEADS, D], f32, tag="pm", name=f"pa{b}")
            for h in range(HEADS):
                nc.tensor.matmul(
                    pa_all[0:D, h, :], g_all[D:P, h, 0:D], dskall[D:P, h, :],
                    start=True, stop=True)
            asb = smp.tile([D, HEADS, D], f32, tag="asb", name=f"asb{b}")
            nc.vector.tensor_mul(asb[:], pa_all[:], rsq[:])
            yield 2.0
            ex = smp.tile([D, HEADS, D], f32, tag="ex", name=f"ex{b}")
            zs = smp.tile([D, HEADS], f32, tag="zs", name=f"zs{b}")
            if no_maxsub:
                nc.scalar.activation(out=ex[:], in_=asb[:], func=AF.Exp)
            else:
                nm = smp.tile([D, HEADS], f32, tag="nm", name=f"nm{b}")
                nc.vector.tensor_reduce(out=nm[:], in_=asb[:],
                                        op=mybir.AluOpType.max, axis=X,
                                        negate=True)
                for h in range(HEADS):
                    nc.scalar.activation(
                        out=ex[:, h, :], in_=asb[:, h, :], func=AF.Exp,
                        bias=nm[:, h:h + 1], scale=1.0)
            nc.vector.tensor_reduce(out=zs[:], in_=ex[:],
                                    op=mybir.AluOpType.add, axis=X)
            rinv = smp.tile([D, HEADS], f32, tag="rinv", name=f"rinv{b}")
            nc.vector.reciprocal(out=rinv[:], in_=zs[:])
            yield 2.0
            # softmax rows -> block-diag tin per head pair -> M
            m_sb = mpool.tile([P, NPAIR, C], bf16, tag="m", name=f"msb{b}")
            tins = []
            for g in range(NPAIR):
                tin = tinp.tile([P, P], f32, tag="tin", name=f"tin{b}_{g}")
                nc.vector.memset(tin[:], 0.0)
                tins.append(tin)
            for g in range(NPAIR):
                tin = tins[g]
                for hh in range(2):
                    h = 2 * g + hh
                    nc.gpsimd.tensor_scalar_mul(
                        out=tin[hh * D:(hh + 1) * D, hh * D:(hh + 1) * D],
                        in0=ex[:, h, :], scalar1=rinv[:, h:h + 1])
                pt = pmid.tile([P, P], f32, tag="pm", name=f"pt{b}_{g}")
                nc.tensor.transpose(pt[:], tin[:], ident[:])
                at2 = atp.tile([P, P], bf16, tag="at", name=f"at{b}_{g}")
                nc.vector.tensor_copy(out=at2[:], in_=pt[:])
                pm = pmid.tile([P, C], f32, tag="pm", name=f"pmm{b}_{g}")
                nc.tensor.matmul(pm[:], at2[:], wvt_sb[:, g, :],
                                 start=True, stop=True)
                nc.vector.tensor_copy(out=m_sb[:, g, :], in_=pm[:])
                yield 2.0
            # Wfused = M^T @ Wp   [C, C]
            wf_sb = wfp.tile([P, CCH, C], bf16, tag="wf", name=f"wfsb{b}")
            for cb in range(CCH):
                pw = pmid.tile([P, C], f32, tag="pm", name=f"pw{b}_{cb}")
                for g in range(NPAIR):
                    nc.tensor.matmul(
                        pw[:], m_sb[:, g, cb * P:(cb + 1) * P], wp_sb[:, g, :],
                        start=(g == 0), stop=(g == NPAIR - 1))
                if cb % 2 == 0:
                    nc.vector.tensor_copy(out=wf_sb[:, cb, :], in_=pw[:])
                else:
                    nc.scalar.copy(out=wf_sb[:, cb, :], in_=pw[:])
                yield 1.0
            st["wf_sb"] = wf_sb

        def gen_YT(b):
            """y^T = Wfused^T X^T over prefetched x^T groups; y written in
            bf16 on the Act queue. Also emits the next batch's x^T loads."""
            st = state[b]
            wf_sb = st["wf_sb"]
            for gi in range(ng):
                if b + 1 < nb:
                    xt_load(b + 1, gi)
                xt_t = st["xt_ts"][gi]
                for co in range(CCH):
                    ysb = ypool.tile([P, 1024], bf16, tag="y",
                                     name=f"ys{b}_{gi}_{co}")
                    for half in range(2):
                        py = pmid.tile([P, C], f32, tag="pm",
                                       name=f"py{b}_{gi}_{co}_{half}")
                        for cb in range(CCH):
                            nc.tensor.matmul(
                                py[:], wf_sb[:, cb, co * P:(co + 1) * P],
                                xt_t[:, cb, half * 512:(half + 1) * 512],
                                start=(cb == 0), stop=(cb == CCH - 1))
                        if half == 0:
                            nc.vector.tensor_copy(
                                out=ysb[:, 0:512], in_=py[:])
                        else:
                            nc.scalar.copy(out=ysb[:, 512:1024], in_=py[:])
                        yield
                    nc.scalar.dma_start(
                        out=y_d[b, co * P:(co + 1) * P,
                                gi * 1024:(gi + 1) * 1024],
                        in_=ysb[:])

        _SENT = object()

        def run(gen):
            for _ in gen:
                pass

        gens_A = [gen_A(b) for b in range(nb)]
        gens_M = [gen_MID(b) for b in range(nb)]
        gens_Y = [gen_YT(b) for b in range(nb)]

        # tensor-dense filler streams, consumed in dependency order
        fillers = []

        def fill(budget):
            while budget > 0 and fillers:
                if next(fillers[0], _SENT) is _SENT:
                    fillers.pop(0)
                else:
                    budget -= 1

        run(gens_A[0])
        frac = [0.0]

        def fill_ratio(r):
            frac[0] += r
            k = int(frac[0])
            frac[0] -= k
            fill(k)

        for b in range(nb):
            if b + 1 < nb:
                fillers.append(gens_A[b + 1])
            for w in gens_M[b]:
                fill_ratio(w or 1.0)
            fillers.append(gens_Y[b])
        while fillers:
            fill(1000)

    nc.compile()
    return nc


def prep_inputs_v3(x, qkv_w, scale, proj_w, n_cores=N_CORES):
    import ml_dtypes

    B, H, W, Cc = x.shape
    assert Cc == C
    n = H * W
    nb = B // n_cores
    nt2 = n // (2 * P)

    xr = np.asarray(x, np.float32).reshape(B, n, C).astype(ml_dtypes.bfloat16)
    # paired tiles: [B, nt2, P, 2, C] so DMA runs are 2 KB per partition
    xbp = np.ascontiguousarray(
        xr.reshape(B, nt2, 2, P, C).transpose(0, 1, 3, 2, 4))
    xt = np.ascontiguousarray(xr.transpose(0, 2, 1))

    w3 = np.asarray(qkv_w, np.float32).reshape(C, HEADS, 3, D)
    wqk = np.ascontiguousarray(w3[:, :, 0:2, :].reshape(C, 2 * C))
    wqk = np.ascontiguousarray(
        wqk.reshape(CCH, P, 2 * C).transpose(1, 0, 2)).astype(ml_dtypes.bfloat16)
    wv = w3[:, :, 2, :].reshape(C, C)
    wvt = np.ascontiguousarray(wv.T)                       # [of, c]
    wvt = np.ascontiguousarray(
        wvt.reshape(NPAIR, P, C).transpose(1, 0, 2)).astype(ml_dtypes.bfloat16)
    wp = np.ascontiguousarray(
        np.asarray(proj_w, np.float32).reshape(NPAIR, P, C).transpose(1, 0, 2)
    ).astype(ml_dtypes.bfloat16)

    es = tuple(float(v) for v in
               np.exp(np.asarray(scale, np.float32)).reshape(HEADS))

    in_maps = []
    for core in range(n_cores):
        in_maps.append({
            "xb": np.ascontiguousarray(xbp[core * nb:(core + 1) * nb]),
            "xt": np.ascontiguousarray(xt[core * nb:(core + 1) * nb]),
            "wqk": wqk, "wvt": wvt, "wp": wp,
        })
    return in_maps, es, (B, H, W, nb, n)



# ---------------------------------------------------------------------------
# v1: per-token qkv fallback (nonzero biases)
# ---------------------------------------------------------------------------

def _build_v1(nb, n, es, add_bqk, add_bv, add_bp):
    from contextlib import ExitStack
    import concourse.bass as bass  # noqa: F401
    from concourse import bacc
    import concourse.mybir as mybir
    import concourse.tile as tile
    from concourse.masks import make_identity

    f32 = mybir.dt.float32
    f32r = mybir.dt.float32r
    bf16 = mybir.dt.bfloat16
    X = mybir.AxisListType.X
    AF = mybir.ActivationFunctionType

    nt = n // P
    nxc = n // 512
    tiles_per_sc = min(8, nt)
    nsc = nt // tiles_per_sc
    xc_per_sc = (512 * nxc) // (512 * nsc)

    nc = bacc.Bacc("TRN2", target_bir_lowering=False)

    xt_d = nc.dram_tensor("xt", [nb, C, n], f32r, kind="ExternalInput")
    wqk_d = nc.dram_tensor("wqk", [P, CCH, 2 * C], f32r, kind="ExternalInput")
    wv_d = nc.dram_tensor("wv", [P, CCH, C], f32r, kind="ExternalInput")
    wp_d = nc.dram_tensor("wp", [P, CCH, C], f32r, kind="ExternalInput")
    y_d = nc.dram_tensor("y", [nb, n, C], f32, kind="ExternalOutput")
    if add_bqk:
        bqk_d = nc.dram_tensor("bqk", [1, 2 * C], f32, kind="ExternalInput")
    if add_bv:
        bv_d = nc.dram_tensor("bv", [C], f32, kind="ExternalInput")
    if add_bp:
        bp_d = nc.dram_tensor("bp", [1, C], f32, kind="ExternalInput")

    with tile.TileContext(nc) as tc, ExitStack() as ctx:
        consts = ctx.enter_context(tc.tile_pool(name="consts", bufs=1))
        vt_pool = ctx.enter_context(tc.tile_pool(name="vt", bufs=1))
        o2_pool = ctx.enter_context(tc.tile_pool(name="o2", bufs=1))
        x_pool = ctx.enter_context(tc.tile_pool(name="xp", bufs=2))
        z_pool = ctx.enter_context(tc.tile_pool(name="zp", bufs=min(9, nt + 1)))
        g_pool = ctx.enter_context(tc.tile_pool(name="gp", bufs=HEADS))
        at_pool = ctx.enter_context(tc.tile_pool(name="atp", bufs=2))
        sm_pool = ctx.enter_context(tc.tile_pool(name="smp", bufs=2))
        y_pool = ctx.enter_context(tc.tile_pool(name="yp", bufs=2))
        pqk = ctx.enter_context(tc.tile_pool(name="pqk", bufs=3, space="PSUM"))
        pgram = ctx.enter_context(tc.tile_pool(name="pgram", bufs=2, space="PSUM"))
        pmisc = ctx.enter_context(tc.tile_pool(name="pmisc", bufs=2, space="PSUM"))
        ptr = ctx.enter_context(tc.tile_pool(name="ptr", bufs=1, space="PSUM"))

        wqk_sb = consts.tile([P, CCH, 2 * C], f32r)
        nc.sync.dma_start(wqk_sb[:], wqk_d[:])
        wv_sb = consts.tile([P, CCH, C], f32r)
        nc.sync.dma_start(wv_sb[:], wv_d[:])
        wp_sb = consts.tile([P, CCH, C], f32r)
        nc.sync.dma_start(wp_sb[:], wp_d[:])
        ident = consts.tile([P, P], f32)
        make_identity(nc, ident[:])
        identb = consts.tile([P, P], bf16)
        nc.vector.tensor_copy(out=identb[:], in_=ident[:])
        ioff = consts.tile([P, D], f32)
        nc.gpsimd.memset(ioff[:], 0.0)
        nc.gpsimd.affine_select(
            out=ioff[:], in_=ioff[:], compare_op=mybir.AluOpType.not_equal,
            fill=1.0, base=-D, pattern=[[-1, D]], channel_multiplier=1,
        )
        if add_bqk:
            bqk_sb = consts.tile([P, 2 * C], f32)
            nc.sync.dma_start(
                out=bqk_sb[:],
                in_=_pbroadcast(bass, bqk_d[:], P),
            )
        if add_bv:
            bv_sb = consts.tile([P, NPAIR], f32)
            nc.sync.dma_start(
                out=bv_sb[:], in_=bv_d[:].rearrange("(g p) -> p g", p=P))
        if add_bp:
            bp_sb = consts.tile([P, C], f32)
            nc.sync.dma_start(
                out=bp_sb[:],
                in_=_pbroadcast(bass, bp_d[:], P),
            )

        for b in range(nb):
            vt = vt_pool.tile([P, NPAIR, n], f32r, tag="vt")
            gsb = [g_pool.tile([P, P], f32, tag="g", name=f"gsb{b}_{h}")
                   for h in range(HEADS)]
            xt_r = xt_d[b].rearrange("(co ci) n -> ci co n", ci=P)

            for sc in range(nsc):
                zs = []
                for xc in range(xc_per_sc):
                    tch = sc * xc_per_sc + xc
                    xt_t = x_pool.tile([P, CCH, 512], f32r, tag="x")
                    nc.sync.dma_start(
                        out=xt_t[:], in_=xt_r[:, :, tch * 512:(tch + 1) * 512])
                    for f in range(NPAIR):
                        pv = pmisc.tile([P, 512], f32, tag="pm")
                        for c in range(CCH):
                            nc.tensor.matmul(
                                pv[:],
                                wv_sb[:, c, f * P:(f + 1) * P],
                                xt_t[:, c, :],
                                start=(c == 0), stop=(c == CCH - 1),
                            )
                        dst = vt[:, f, tch * 512:(tch + 1) * 512]
                        if add_bv:
                            nc.vector.tensor_scalar(
                                out=dst, in0=pv[:], scalar1=bv_sb[:, f:f + 1],
                                scalar2=None, op0=mybir.AluOpType.add)
                        else:
                            nc.vector.tensor_copy(out=dst, in_=pv[:])
                    for t4 in range(4):
                        z = z_pool.tile([P, 2 * C], bf16, tag="z")
                        for fc in range(2):
                            pq = pqk.tile([P, 512], f32, tag="pq")
                            for c in range(CCH):
                                nc.tensor.matmul(
                                    pq[:],
                                    xt_t[:, c, t4 * P:(t4 + 1) * P],
                                    wqk_sb[:, c, fc * 512:(fc + 1) * 512],
                                    start=(c == 0), stop=(c == CCH - 1),
                                )
                            zdst = z[:, fc * 512:(fc + 1) * 512]
                            if add_bqk:
                                nc.vector.tensor_add(
                                    out=zdst, in0=pq[:],
                                    in1=bqk_sb[:, fc * 512:(fc + 1) * 512])
                            else:
                                nc.vector.tensor_copy(out=zdst, in_=pq[:])
                        zs.append(z)
                for h in range(HEADS):
                    pg = pgram.tile([P, P], f32, tag="pg")
                    for i, z in enumerate(zs):
                        zh = z[:, h * P:(h + 1) * P]
                        nc.tensor.matmul(
                            pg[:], zh, zh,
                            start=(i == 0), stop=(i == len(zs) - 1))
                    if sc == 0:
                        nc.vector.tensor_copy(out=gsb[h][:], in_=pg[:])
                    else:
                        nc.vector.tensor_add(
                            out=gsb[h][:], in0=gsb[h][:], in1=pg[:])

            o2 = o2_pool.tile([P, NPAIR, n], f32r, tag="o2")
            for g in range(NPAIR):
                tin = sm_pool.tile([P, P], f32, tag="tin")
                nc.vector.memset(tin[:], 0.0)
                for hh in range(2):
                    h = 2 * g + hh
                    G = gsb[h]
                    dtmp = sm_pool.tile([P, P], f32, tag="dtmp")
                    nc.vector.tensor_mul(dtmp[:], G[:], ident[:])
                    s = sm_pool.tile([P, 1], f32, tag="s")
                    nc.vector.reduce_sum(out=s[:], in_=dtmp[:], axis=X)
                    nc.vector.tensor_scalar_max(out=s[:], in0=s[:], scalar1=EPS)
                    srt = sm_pool.tile([P, 1], f32, tag="srt")
                    nc.scalar.activation(out=srt[:], in_=s[:], func=AF.Sqrt)
                    nc.vector.reciprocal(out=s[:], in_=srt[:])
                    if es[h] != 1.0:
                        nc.scalar.mul(out=s[0:D, :], in_=s[0:D, :], mul=es[h])
                    dsk = sm_pool.tile([P, D], f32, tag="dsk")
                    nc.vector.tensor_scalar_mul(
                        out=dsk[D:P, :], in0=ioff[D:P, :], scalar1=s[D:P, :])
                    pa = ptr.tile([P, P], f32, tag="pt")
                    nc.tensor.matmul(
                        pa[0:D, 0:D],
                        G[D:P, 0:D],
                        dsk[D:P, :],
                        start=True, stop=True,
                    )
                    asb = sm_pool.tile([D, D], f32, tag="asb")
                    nc.vector.tensor_scalar_mul(
                        out=asb[:], in0=pa[0:D, 0:D], scalar1=s[0:D, :])
                    nm = sm_pool.tile([D, 1], f32, tag="nm")
                    nc.vector.tensor_reduce(
                        out=nm[:], in_=asb[:], op=mybir.AluOpType.max,
                        axis=X, negate=True)
                    ex = sm_pool.tile([D, D], f32, tag="ex")
                    zsum = sm_pool.tile([D, 1], f32, tag="zsum")
                    nc.scalar.activation(
                        out=ex[:], in_=asb[:], func=AF.Exp,
                        bias=nm[:], scale=1.0, accum_out=zsum[:])
                    rinv = sm_pool.tile([D, 1], f32, tag="rinv")
                    nc.vector.reciprocal(out=rinv[:], in_=zsum[:])
                    nc.vector.tensor_scalar_mul(
                        out=tin[hh * D:(hh + 1) * D, hh * D:(hh + 1) * D],
                        in0=ex[:], scalar1=rinv[:])
                pt = ptr.tile([P, P], f32, tag="pt")
                nc.tensor.transpose(pt[:], tin[:], ident[:])
                at2 = at_pool.tile([P, P], f32r, tag="at")
                nc.vector.tensor_copy(out=at2[:], in_=pt[:])
                for ch in range(n // 512):
                    po = pmisc.tile([P, 512], f32, tag="pm")
                    nc.tensor.matmul(
                        po[:],
                        at2[:],
                        vt[:, g, ch * 512:(ch + 1) * 512],
                        start=True, stop=True,
                    )
                    nc.scalar.copy(
                        out=o2[:, g, ch * 512:(ch + 1) * 512], in_=po[:])

            for tt in range(nt):
                py = pmisc.tile([P, 512], f32, tag="pm")
                for g in range(NPAIR):
                    nc.tensor.matmul(
                        py[:],
                        o2[:, g, tt * P:(tt + 1) * P],
                        wp_sb[:, g, :],
                        start=(g == 0), stop=(g == NPAIR - 1),
                    )
                ysb = y_pool.tile([P, C], f32, tag="y")
                if add_bp:
                    nc.vector.tensor_add(out=ysb[:], in0=py[:], in1=bp_sb[:])
                else:
                    nc.vector.tensor_copy(out=ysb[:], in_=py[:])
                nc.sync.dma_start(
                    out=y_d[b, tt * P:(tt + 1) * P, :], in_=ysb[:])

    nc.compile()
    return nc


def prep_inputs_v1(x, qkv_w, q_bias, v_bias, scale, proj_w, proj_b,
                   n_cores=N_CORES):
    B, H, W, Cc = x.shape
    assert Cc == C
    n = H * W
    nb = B // n_cores

    xt = np.ascontiguousarray(
        x.reshape(B, n, C).transpose(0, 2, 1)).astype(np.float32, copy=False)

    w3 = qkv_w.reshape(C, HEADS, 3, D)
    wqk = np.ascontiguousarray(w3[:, :, 0:2, :].reshape(C, 2 * C))
    wv = np.ascontiguousarray(w3[:, :, 2, :].reshape(C, C))
    wqk = np.ascontiguousarray(wqk.reshape(CCH, P, 2 * C).transpose(1, 0, 2))
    wv = np.ascontiguousarray(wv.reshape(CCH, P, C).transpose(1, 0, 2))
    wp = np.ascontiguousarray(proj_w.reshape(CCH, P, C).transpose(1, 0, 2))

    bias_full = np.concatenate(
        [q_bias, np.zeros_like(q_bias), v_bias]).astype(np.float32)
    b3 = bias_full.reshape(HEADS, 3, D)
    bqk = np.ascontiguousarray(b3[:, 0:2, :].reshape(1, 2 * C))
    bv = np.ascontiguousarray(b3[:, 2, :].reshape(C))
    bp = np.asarray(proj_b, np.float32).reshape(1, C)

    add_bqk = bool(np.any(bqk))
    add_bv = bool(np.any(bv))
    add_bp = bool(np.any(bp))
    es = tuple(float(v) for v in
               np.exp(np.asarray(scale, np.float32)).reshape(HEADS))

    in_maps = []
    for core in range(n_cores):
        m = {
            "xt": np.ascontiguousarray(xt[core * nb:(core + 1) * nb]),
            "wqk": wqk, "wv": wv, "wp": wp,
        }
        if add_bqk:
            m["bqk"] = bqk
        if add_bv:
            m["bv"] = bv
        if add_bp:
            m["bp"] = bp
        in_maps.append(m)
    return in_maps, es, (add_bqk, add_bv, add_bp), (B, H, W, nb, n)


def _get_nc(key, builder, *args):
    if key not in _CACHE:
        _CACHE[key] = builder(*args)
    return _CACHE[key]


def kernel(x, qkv_w, q_bias, v_bias, scale, proj_w, proj_b):
    from concourse.bass_utils import run_bass_kernel_spmd

    trace = bool(int(os.environ.get("KERNEL_TRACE", "0")))
    zero_bias = not (np.any(q_bias) or np.any(v_bias) or np.any(proj_b))
    B, H, W, _ = x.shape

    if zero_bias:
        in_maps, es, (B, H, W, nb, n) = prep_inputs_v3(x, qkv_w, scale, proj_w)
        nc = _get_nc(("v3", nb, n, es), _build_v3, nb, n, es)
        res = run_bass_kernel_spmd(
            nc, in_maps, core_ids=list(range(N_CORES)), trace=trace)
        yt = np.concatenate(
            [np.asarray(r["y"], np.float32) for r in res.results], axis=0)
        out = np.ascontiguousarray(yt.transpose(0, 2, 1)).reshape(B, H, W, C)
    else:
        in_maps, es, gates, (B, H, W, nb, n) = prep_inputs_v1(
            x, qkv_w, q_bias, v_bias, scale, proj_w, proj_b)
        nc = _get_nc(("v1", nb, n, es, gates), _build_v1, nb, n, es, *gates)
        res = run_bass_kernel_spmd(
            nc, in_maps, core_ids=list(range(N_CORES)), trace=trace)
        y = np.concatenate([r["y"] for r in res.results], axis=0)
        out = y.reshape(B, H, W, C)

    out = out.astype(np.float32, copy=False)
    kernel.last_results = res
    return out

